# revision 14
# baseline (speedup 1.0000x reference)
"""MoE SwiGLU expert kernel for Trainium2, 8 NeuronCores — all-bf16.

Problem: x[4,2048,4096] routed through K=4 active experts (of 16):
    g = x @ gate[k], u = x @ up[k], act = silu(g)*u, out = act @ down[k]
    out[b,s,k,h], inputs float32, accuracy gate rel_err < 2e-2.

Sharding (8 cores): 4-way over tokens x 2-way over the expert hidden dim E.
  core c -> (tau = c//2: tokens [2048*tau, 2048*tau+2048),
             eps = c%2:  E-half [896*eps, 896*eps+896) of every active expert)
Each core computes a partial down-projection summed over its E-half; host
adds the two partials (written bf16) of each token quarter.

Why bf16: fp32r and bf16 both run 1 row/cycle on the PE, but measured
per-matmul stride is ~221ns for bf16 with 4 PSUM subtiles per stationary
vs ~233.5ns for fp32r (which is SBUF-bound to 2 subtiles: fp32 x for 2048
tokens would need 256KB/partition). LDWEIGHTS issue is fully hidden by
the PE weight shadow register either way (verified: stride is identical
after an LDW vs within a pair). bf16 also halves every DMA stream: x
(16MB) and all weights fit one pass — total HBM traffic drops from
~418MB to ~124MB per core, eliminating the block-transition bandwidth
crunch entirely. Accuracy cost of bf16 inputs/act is ~3e-3 rel, well
under the 2e-2 gate. fp8 (2x PE rate) fails the gate (~4-7e-2).
NOTE: ldw-opt must stay DISABLED — bf16's explicit Ldweights is
incompatible with that walrus pass (and elision is unnecessary anyway).

Single block: all 2048 tokens resident in SBUF as 16 h-pair slabs
[128,2,2048] bf16 (8KB/partition each). PSUM: one 8-bank pool — g/u uses
psg[4]+psu[4] (4 subtiles of 512 per stationary, s-inner so the repeated
stationary needs no reload); down uses pso[4] which rotate into the freed
g/u banks and double-buffer across i-tiles.

DMA queues (sync/scalar/gpsimd): g/u window: gate halves [128,16,128] on
sync, up halves on scalar. Down window: dch [128,7,128] on sync, out
[128,2048] bf16 on gpsimd, next-expert j0 halves on scalar; dch i0/i1
prefetched during j6. Cold start: gw half m0 leads sync, x pair p0 leads
scalar, gpsimd streams the odd pairs; ~18MB total at ~358GB/s aggregate
paces the first expert's j0 chain. The final out tile is shipped
per-subtile across scalar+gpsimd to cut the end drain.
"""
import functools
import sys

sys.path.insert(0, "/opt/trn_rl_repo")

import numpy as np
import ml_dtypes

import concourse.bass as bass
import concourse.mybir as mybir
import concourse.tile as tile
from concourse import bacc
from concourse.bass_utils import run_bass_kernel_spmd

F32 = mybir.dt.float32
BF16 = mybir.dt.bfloat16

B, S, H, E, NEXP, K = 4, 2048, 4096, 1792, 16, 4
N_CORES = 8
TOK = B * S                  # 8192 tokens
TOK_PC = TOK // 4            # 2048 tokens per core (4-way token split)
E_PC = E // 2                # 896 e-channels per core (2-way E split)
N_ET = E_PC // 128           # 7 e-tiles
N_HT = H // 128              # 32 h-tiles
TBLK = TOK_PC                # all 2048 tokens resident (single block)
TSUB = 512                   # PSUM moving free dim
N_TSUB = TBLK // TSUB        # 4 subtiles per stationary
HH_HALF = N_HT // 2          # h-tiles per weight half-chunk (16)
XP = 2                       # h-tiles per x slab pair
N_XP = N_HT // XP            # 16 x slabs

BF = ml_dtypes.bfloat16


def _build(n_experts=K, n_etiles=N_ET, n_htiles=N_HT):
    nc = bacc.Bacc(
        "TRN2",
        target_bir_lowering=False,
        debug=False,
        enable_asserts=False,
        num_devices=N_CORES,
    )
    e_pc = n_etiles * 128
    h_full = n_htiles * 128
    xT = nc.dram_tensor("xT", [h_full, TOK_PC], BF16, kind="ExternalInput")
    gw = nc.dram_tensor("gw", [n_experts, h_full, e_pc], BF16, kind="ExternalInput")
    uw = nc.dram_tensor("uw", [n_experts, h_full, e_pc], BF16, kind="ExternalInput")
    dw = nc.dram_tensor("dw", [n_experts, e_pc, h_full], BF16, kind="ExternalInput")
    out = nc.dram_tensor("out", [n_experts, h_full, TOK_PC], BF16, kind="ExternalOutput")

    silu = mybir.ActivationFunctionType.Silu

    with tile.TileContext(nc) as tc:
        with (
            tc.tile_pool(name="xpool", bufs=N_XP) as xpool,
            tc.tile_pool(name="gupool", bufs=4) as gupool,
            tc.tile_pool(name="dpool", bufs=3) as dpool,
            tc.tile_pool(name="actpool", bufs=n_etiles) as actpool,
            tc.tile_pool(name="silpool", bufs=2) as silpool,
            tc.tile_pool(name="opool", bufs=4) as opool,
            tc.tile_pool(name="psum", bufs=8, space="PSUM") as psum,
        ):
            def emit_x(p, eng):
                xsl = xpool.tile([128, XP, TBLK], BF16, tag="x", name="xsl")
                h0 = p * XP * 128
                eng.dma_start(
                    out=xsl,
                    in_=xT[h0 : h0 + XP * 128, :].rearrange(
                        "(i p) t -> p i t", p=128
                    ),
                )
                return xsl

            def emit_wch(wdram, k, j, m, eng):
                wch = gupool.tile([128, HH_HALF, 128], BF16, tag="gu", name="wch")
                h0 = m * HH_HALF * 128
                e0 = j * 128
                eng.dma_start(
                    out=wch,
                    in_=wdram[
                        k, h0 : h0 + HH_HALF * 128, e0 : e0 + 128
                    ].rearrange("(hh p) e -> p hh e", p=128),
                )
                return wch

            def emit_dch(k, i, eng):
                dch = dpool.tile([128, n_etiles, 128], BF16, tag="d", name="dch")
                eng.dma_start(
                    out=dch,
                    in_=dw[k, :, i * 128 : (i + 1) * 128].rearrange(
                        "(j p) h -> p j h", p=128
                    ),
                )
                return dch

            # cold start, byte-balanced in rough need-order:
            #   sync:   gw m0, p2, gw m1, p5, p8, p11, p14   (2MB w + 5MB x)
            #   scalar: p0, uw m0, p3, uw m1, p6, p9, p12, p15 (2MB w + 6MB x)
            #   gpsimd: p1, p4, p7, p10, p13                 (5MB x)
            xslabs = [None] * N_XP
            xslabs[0] = emit_x(0, nc.scalar)
            xslabs[1] = emit_x(1, nc.gpsimd)
            wg = [emit_wch(gw, 0, 0, 0, nc.sync)]
            wu = [emit_wch(uw, 0, 0, 0, nc.scalar)]
            xslabs[2] = emit_x(2, nc.sync)
            xslabs[3] = emit_x(3, nc.scalar)
            xslabs[4] = emit_x(4, nc.gpsimd)
            wg.append(emit_wch(gw, 0, 0, 1, nc.sync))
            wu.append(emit_wch(uw, 0, 0, 1, nc.scalar))
            for p, eng in (
                (5, nc.sync), (6, nc.scalar), (7, nc.gpsimd),
                (8, nc.sync), (9, nc.scalar), (10, nc.gpsimd),
                (11, nc.sync), (12, nc.scalar), (13, nc.gpsimd),
                (14, nc.sync), (15, nc.scalar),
            ):
                xslabs[p] = emit_x(p, eng)
            pre = (wg, wu)

            def xts_at(hi):
                return xslabs[hi // XP][:, hi % XP, :]

            for k in range(n_experts):
                act_tiles = []
                dch_pre = []
                for j in range(n_etiles):
                    if pre is not None:
                        wg, wu = pre
                        pre = None
                    else:
                        wg = [emit_wch(gw, k, j, m, nc.sync) for m in range(2)]
                        wu = [emit_wch(uw, k, j, m, nc.scalar) for m in range(2)]
                    if j == n_etiles - 1:
                        # first down chunks prefetched while the last
                        # e-tile's g/u matmuls still run
                        dch_pre = [emit_dch(k, i, nc.sync) for i in range(2)]
                    psg = [
                        psum.tile([128, TSUB], F32, tag="ps", name=f"psg{s}")
                        for s in range(N_TSUB)
                    ]
                    psu = [
                        psum.tile([128, TSUB], F32, tag="ps", name=f"psu{s}")
                        for s in range(N_TSUB)
                    ]
                    if k == 0 and j == 0:
                        # cold start: interleave g/u per h-tile so each x pair
                        # is needed at half the rate while x still streams in
                        for hi in range(n_htiles):
                            m, r = divmod(hi, HH_HALF)
                            for wch, ps in ((wg, psg), (wu, psu)):
                                for s in range(N_TSUB):
                                    nc.tensor.matmul(
                                        ps[s],
                                        wch[m][:, r, :],
                                        xts_at(hi)[:, s * TSUB : (s + 1) * TSUB],
                                        start=(hi == 0),
                                        stop=(hi == n_htiles - 1),
                                    )
                    else:
                        for wch, ps in ((wg, psg), (wu, psu)):
                            for hi in range(n_htiles):
                                m, r = divmod(hi, HH_HALF)
                                # s-inner: all 4 subtiles share this stationary
                                for s in range(N_TSUB):
                                    nc.tensor.matmul(
                                        ps[s],
                                        wch[m][:, r, :],
                                        xts_at(hi)[:, s * TSUB : (s + 1) * TSUB],
                                        start=(hi == 0),
                                        stop=(hi == n_htiles - 1),
                                    )
                    act_j = actpool.tile([128, TBLK], BF16, tag="act", name="act_j")
                    for s in range(N_TSUB):
                        sil = silpool.tile([128, TSUB], F32, tag="sil", name="sil")
                        nc.scalar.activation(sil, psg[s], silu)
                        nc.vector.tensor_mul(
                            act_j[:, s * TSUB : (s + 1) * TSUB], sil, psu[s]
                        )
                    act_tiles.append(act_j)

                # ---- down phase ----
                # dch streams on sync; out (bf16) on gpsimd; scalar carries
                # the next expert's j=0 halves
                last_k = k == n_experts - 1
                if not last_k:
                    pre = (
                        [emit_wch(gw, k + 1, 0, m, nc.scalar) for m in range(2)],
                        [emit_wch(uw, k + 1, 0, m, nc.scalar) for m in range(2)],
                    )
                for i in range(n_htiles):
                    dch = (dch_pre[i] if i < 2
                           else emit_dch(k, i, nc.sync))
                    pso = [
                        psum.tile([128, TSUB], F32, tag="ps", name=f"pso{s}")
                        for s in range(N_TSUB)
                    ]
                    split_last = last_k and i == n_htiles - 1
                    if split_last:
                        # tail: s-outer so each subtile's chain finishes 7
                        # matmuls before the next — its copy + per-subtile
                        # out DMA overlap the remaining chains instead of
                        # all draining after the final matmul
                        ot = opool.tile([128, TBLK], BF16, tag="ot", name="ot")
                        for s in range(N_TSUB):
                            for j in range(n_etiles):
                                nc.tensor.matmul(
                                    pso[s],
                                    dch[:, j, :],
                                    act_tiles[j][:, s * TSUB : (s + 1) * TSUB],
                                    start=(j == 0),
                                    stop=(j == n_etiles - 1),
                                )
                            nc.vector.tensor_copy(
                                ot[:, s * TSUB : (s + 1) * TSUB], pso[s]
                            )
                            eng = nc.scalar if s % 2 == 0 else nc.gpsimd
                            eng.dma_start(
                                out=out[
                                    k,
                                    i * 128 : (i + 1) * 128,
                                    s * TSUB : (s + 1) * TSUB,
                                ],
                                in_=ot[:, s * TSUB : (s + 1) * TSUB],
                            )
                    else:
                        for j in range(n_etiles):
                            # s-inner: all 4 subtiles share the dch stationary
                            for s in range(N_TSUB):
                                nc.tensor.matmul(
                                    pso[s],
                                    dch[:, j, :],
                                    act_tiles[j][:, s * TSUB : (s + 1) * TSUB],
                                    start=(j == 0),
                                    stop=(j == n_etiles - 1),
                                )
                        ot = opool.tile([128, TBLK], BF16, tag="ot", name="ot")
                        for s in range(N_TSUB):
                            nc.vector.tensor_copy(
                                ot[:, s * TSUB : (s + 1) * TSUB], pso[s]
                            )
                        eng = (nc.scalar if last_k and i >= 28 and i % 2 == 0
                               else nc.gpsimd)
                        eng.dma_start(
                            out=out[k, i * 128 : (i + 1) * 128, :],
                            in_=ot,
                        )
    nc.compile()
    return nc


@functools.cache
def _built_full():
    return _build()


def kernel(x, gate_proj, up_proj, down_proj, expert_idx):
    x = np.asarray(x)
    idx = np.asarray(expert_idx)
    gate = np.asarray(gate_proj)[idx]  # [K, H, E]
    up = np.asarray(up_proj)[idx]
    down = np.asarray(down_proj)[idx]  # [K, E, H]

    nc = _built_full()

    xf = x.reshape(TOK, H)
    in_maps = []
    for c in range(N_CORES):
        tau, eps = divmod(c, 2)
        tsl = slice(TOK_PC * tau, TOK_PC * (tau + 1))
        esl = slice(E_PC * eps, E_PC * (eps + 1))
        in_maps.append(
            {
                "xT": np.ascontiguousarray(xf[tsl].T).astype(BF),
                "gw": np.ascontiguousarray(gate[:, :, esl]).astype(BF),
                "uw": np.ascontiguousarray(up[:, :, esl]).astype(BF),
                "dw": np.ascontiguousarray(down[:, esl, :]).astype(BF),
            }
        )

    res = run_bass_kernel_spmd(nc, in_maps, core_ids=list(range(N_CORES)))

    out = np.empty((TOK, K, H), dtype=np.float32)
    for tau in range(4):
        part = res.results[2 * tau]["out"].astype(np.float32) + res.results[
            2 * tau + 1
        ]["out"].astype(np.float32)
        # part: [K, H, TOK_PC] -> [TOK_PC, K, H]
        out[TOK_PC * tau : TOK_PC * (tau + 1)] = part.transpose(2, 0, 1)
    return out.reshape(B, S, K, H)
